# revision 1
# baseline (speedup 1.0000x reference)
"""Trainium2 Bass kernel for the JointLoss problem (contrastive NT-Xent + 2 MSE terms).

kernel(representation, xrecon, xorig) -> (loss, closs, recon_loss, zrecon_loss)

Strategy (8 NeuronCores, SPMD — one NEFF, per-core variation only via inputs):
  - closs: sim = z@z.T with z = r/||r||.  Fold the normalization and 1/tau into
    per-column scales s_j = 1/(||r_j|| sqrt(tau)): each core computes a
    (512, 4096) slab of logits from bf16 R^T tiles, applies the per-row scale
    inside the fused exp (activation scale AP), and accumulates row sums with
    the activation accumulator.  Positives come from the diagonal of the
    partner block; the self-similarity term is the constant e^(1/tau).
  - Column chunks of R^T are permuted per core so chunk0 = partner block and
    chunk1 = own slab, making the kernel core-id independent.
  - The scale pipeline runs at 512-column granularity so the first GEMM
    starts early: DMA half -> DVE squares -> PE per-column-block reduce
    matmuls (partition layout [128,4]) -> one ln+exp pair -> M=1 transpose
    matmuls -> free-dim copy -> GPSIMD partition broadcast -> DVE scale mult.
  - MSE partials: bf16 inputs, subtract on DVE, square+accumulate on Act.
  - All partials reduced over partitions with one fp32 matmul -> (10,1)/core;
    host sums the 8 cores' partials.
  - A single activation-function table (natural_log_exp_and_others) serves
    exp/ln/copy, avoiding 1.3us table reloads between functions.
"""

import math

import ml_dtypes
import numpy as np

TAU = 0.5
N = 2048
TWO_N = 4096
D = 512
NCORES = 8
CH = 512  # column chunk (one per core-slab)

_CACHE = {}


def _build_nc():
    import concourse.bacc as bacc
    import concourse.mybir as mybir
    import concourse.tile as tile
    from concourse.masks import make_identity

    F32 = mybir.dt.float32
    BF16 = mybir.dt.bfloat16
    AX = mybir.AxisListType
    OP = mybir.AluOpType
    AF = mybir.ActivationFunctionType

    nc = bacc.Bacc("TRN2", target_bir_lowering=False, debug=False)
    # rt[ch][p][d*512+c] = R^T[d*128+p, 512*ch + c] (permuted cols, 5 chunks)
    rt = nc.dram_tensor("rt", [5, 128, 2048], BF16, kind="ExternalInput")
    xr = nc.dram_tensor("xr", [128, 4096], BF16, kind="ExternalInput")
    xo = nc.dram_tensor("xo", [128, 4096], BF16, kind="ExternalInput")
    zi = nc.dram_tensor("zi", [128, 1024], BF16, kind="ExternalInput")
    zj = nc.dram_tensor("zj", [128, 1024], BF16, kind="ExternalInput")
    out = nc.dram_tensor("out", [128, 60], F32, kind="ExternalOutput")
    orow = nc.dram_tensor("orow", [128, 8], F32, kind="ExternalOutput")
    ocol = nc.dram_tensor("ocol", [1, 1536], F32, kind="ExternalOutput")

    EXP_DIAG = math.exp(1.0 / TAU)

    with tile.TileContext(nc) as tc:
        with (
            tc.tile_pool(name="singles", bufs=1) as singles,
            tc.tile_pool(name="sqp", bufs=5) as sqp,
            tc.tile_pool(name="smallp", bufs=4) as smallp,
            tc.tile_pool(name="msep", bufs=2) as msep,
            tc.tile_pool(name="expsb", bufs=2) as expsb,
            tc.tile_pool(name="mpsum", bufs=2, space="PSUM") as mpsum,
            tc.tile_pool(name="spsum", bufs=1, space="PSUM") as spsum,
            tc.tile_pool(name="rpsum", bufs=1, space="PSUM") as rpsum,
            tc.tile_pool(name="capsum", bufs=1, space="PSUM") as capsum,
        ):
            identf = singles.tile([128, 128], F32, tag="identf")
            make_identity(nc, identf)
            identb = singles.tile([128, 128], BF16, tag="identb")
            make_identity(nc, identb)
            ones_k = singles.tile([128, 1], BF16, tag="ones_k")
            nc.vector.memset(ones_k, 1.0)
            ones_f = singles.tile([128, 1], F32, tag="ones_f")
            nc.vector.memset(ones_f, 1.0)
            s_bcast = singles.tile([128, 1, 2560], BF16, tag="s_bcast")
            srow = singles.tile([1, 2560], BF16, tag="srow")
            eacc = singles.tile([128, 4, 3], F32, tag="eacc")
            orowt = singles.tile([128, 8], F32, tag="orowt")
            ocolt = singles.tile([1, 1536], F32, tag="ocolt")
            obnt = singles.tile([128, 5, 2, 6], F32, tag="obnt")

            # DMA issue spread: SP gets chunks 0/1/4, Pool gets 2/3
            rt_t = []
            rts_t = []
            s32f = []
            for ch in range(5):
                t = singles.tile([128, 4, 512], BF16, tag=f"rt_{ch}")
                eng = nc.sync if ch in (0, 1, 4) else nc.gpsimd
                if ch < 2:
                    # split the first two chunks so their norm chains start
                    # as soon as the first half of the transfer lands
                    eng.dma_start(t[:, 0:2, :], rt[ch][:, 0:1024])
                    eng.dma_start(t[:, 2:4, :], rt[ch][:, 1024:2048])
                else:
                    eng.dma_start(t, rt[ch])
                rt_t.append(t)
                t2 = singles.tile([128, 4, 512], BF16, tag=f"rts_{ch}")
                rts_t.append(t2)
                t3 = singles.tile([128, 4], F32, tag=f"s32f_{ch}")
                s32f.append(t3)

            sq_t = {}

            def prep_sq(ch, on_act):
                # squares for the late chunks run on Act, which is idle
                # through the prefix; the DVE keeps only ch0/ch1 + the
                # scale multiplies, shortening its saturated window
                sq = sqp.tile([128, 4, 512], BF16, tag="sq")
                if on_act:
                    nc.scalar.activation(sq, rt_t[ch], AF.Square)
                elif ch < 2:
                    nc.vector.tensor_tensor(
                        sq[:, 0:2, :], rt_t[ch][:, 0:2, :], rt_t[ch][:, 0:2, :], OP.mult
                    )
                    nc.vector.tensor_tensor(
                        sq[:, 2:4, :], rt_t[ch][:, 2:4, :], rt_t[ch][:, 2:4, :], OP.mult
                    )
                else:
                    nc.vector.tensor_tensor(sq, rt_t[ch], rt_t[ch], OP.mult)
                sq_t[ch] = sq

            def prep(ch):
                sq = sq_t[ch]
                ps = spsum.tile([128, 4], F32, tag="s32ps")
                for b in range(4):
                    for d in range(4):
                        nc.tensor.matmul(
                            ps[:, b : b + 1],
                            sq[:, d, 128 * b : 128 * (b + 1)],
                            ones_k,
                            start=(d == 0),
                            stop=(d == 3),
                        )
                # s = exp(-0.5 * ln(tau * sumsq)) = 1/(sqrt(tau)*||r||)
                lnb = smallp.tile([128, 4], F32, tag="lnb")
                nc.scalar.activation(lnb, ps, AF.Ln, scale=TAU)
                nc.scalar.activation(s32f[ch], lnb, AF.Exp, scale=-0.5)
                s32b = smallp.tile([128, 4], BF16, tag="s32b")
                nc.scalar.activation(s32b, lnb, AF.Exp, scale=-0.5)
                # to free-dim layout: M=1 transpose matmuls, copy, pool bcast
                rp = rpsum.tile([1, 512], F32, tag="rp")
                for b in range(4):
                    nc.tensor.matmul(
                        rp[0:1, 128 * b : 128 * (b + 1)],
                        s32b[:, b : b + 1],
                        identb,
                        start=True,
                        stop=True,
                    )
                off = 512 * ch
                nc.scalar.copy(srow[0:1, off : off + 512], rp)
                nc.gpsimd.partition_broadcast(
                    s_bcast[:, 0, off : off + 512], srow[0:1, off : off + 512]
                )
                nc.vector.tensor_tensor(
                    rts_t[ch],
                    rt_t[ch],
                    s_bcast[:, 0:1, off : off + 512].broadcast_to((128, 4, 512)),
                    OP.mult,
                )

            def main_block(blk):
                # blk 0: chunks 0+1 (partner+own, row-acc only, in-place exp)
                # blk 1: chunks 2+3; blk 2: chunk 4 (sbuf exp + column sums)
                chunks = [(0, 1), (2, 3), (4,)][blk]
                width = 512 * len(chunks)
                cas = []
                if blk > 0:
                    for half in range(len(chunks)):
                        cat = capsum.tile([1, 512], F32, tag=f"ca_{half}")
                        cas.append(cat)
                for rr in range(4):
                    ps = mpsum.tile([128, 1024], F32, tag="mps")
                    for half, ch in enumerate(chunks):
                        for d in range(4):
                            nc.tensor.matmul(
                                ps[:, CH * half : CH * (half + 1)],
                                rt_t[1][:, d, 128 * rr : 128 * (rr + 1)],
                                rts_t[ch][:, d, :],
                                start=(d == 0),
                                stop=(d == 3),
                            )
                    if blk == 0:
                        # positives: diagonal of the partner block (cols 0..511)
                        ext = smallp.tile([128, 128], F32, tag="ext")
                        nc.vector.tensor_tensor(
                            ext, ps[:, 128 * rr : 128 * (rr + 1)], identf, OP.mult
                        )
                        posr = smallp.tile([128, 1], F32, tag="posr")
                        nc.vector.reduce_sum(posr, ext, axis=AX.X)
                        nc.vector.tensor_tensor(
                            orowt[:, 4 + rr : 5 + rr],
                            posr,
                            s32f[1][:, rr : rr + 1],
                            OP.mult,
                        )
                        nc.scalar.activation(
                            ps,
                            ps,
                            AF.Exp,
                            scale=s32f[1][:, rr : rr + 1],
                            accum_out=eacc[:, rr, 0:1],
                        )
                    else:
                        eb = expsb.tile([128, 1024], BF16, tag="eb")
                        nc.scalar.activation(
                            eb[:, 0:width],
                            ps[:, 0:width],
                            AF.Exp,
                            scale=s32f[1][:, rr : rr + 1],
                            accum_out=eacc[:, rr, blk : blk + 1],
                        )
                        for half, ch in enumerate(chunks):
                            nc.tensor.matmul(
                                cas[half],
                                ones_k,
                                eb[:, CH * half : CH * (half + 1)],
                                start=(rr == 0),
                                stop=(rr == 3),
                            )
                if blk > 0:
                    for half, ch in enumerate(chunks):
                        nc.scalar.copy(
                            ocolt[0:1, 512 * (ch - 2) : 512 * (ch - 1)], cas[half]
                        )

            xrt = singles.tile([128, 4096], BF16, tag="xrt")
            nc.sync.dma_start(xrt, xr[:, :])
            xot = singles.tile([128, 4096], BF16, tag="xot")
            nc.sync.dma_start(xot, xo[:, :])
            zit = singles.tile([128, 1024], BF16, tag="zit")
            nc.sync.dma_start(zit, zi[:, :])
            zjt = singles.tile([128, 1024], BF16, tag="zjt")
            nc.sync.dma_start(zjt, zj[:, :])

            def mse(t):
                # subtract on DVE; one-pass count/mean/M2 via bn_stats,
                # sum-of-squares reconstructed on the host
                if t < 4:
                    a = xrt[:, 1024 * t : 1024 * (t + 1)]
                    b = xot[:, 1024 * t : 1024 * (t + 1)]
                else:
                    a, b = zit, zjt
                dx = msep.tile([128, 2, 512], BF16, tag="dx")
                nc.vector.tensor_tensor(dx, a, b, OP.subtract)
                for s in range(2):
                    nc.vector.bn_stats(obnt[:, t, s], dx[:, s])

            # emission order: prep chunks 0-2 early, B0 before preps 3/4 so
            # the Act/PE queues stay decoupled from the late scale chain
            prep_sq(0, False)
            prep(0)
            prep_sq(1, False)
            prep(1)
            prep_sq(2, False)
            prep_sq(3, False)
            prep_sq(4, False)
            main_block(0)
            prep(2)
            prep(3)
            prep(4)
            main_block(1)
            main_block(2)
            for t in range(5):
                mse(t)

            # per-row partial sums (ln happens on the host after the
            # cross-core column-partial reduction)
            nc.vector.tensor_reduce(
                orowt[:, 0:4], eacc[:, :, :], axis=AX.X, op=OP.add
            )
            nc.sync.dma_start(orow[:, :], orowt)
            nc.sync.dma_start(ocol[:, :], ocolt)
            nc.sync.dma_start(out[:, :], obnt)

    # Force a single activation-function table: every function this kernel
    # uses (exp, ln, copy) lives in the natural_log_exp_and_others set, but
    # the load-insertion pass greedily picks the first set per function and
    # thrashes.  Emptying every other set (indices preserved) makes the pass
    # emit one hoisted load.
    import concourse.bacc as bacc_mod
    from concourse.hw_specs import get_activation_tables

    real = get_activation_tables(nc.m.arch)
    target = "natural_log_exp_and_others"
    assert target in real
    filtered = {k: (v if k == target else set()) for k, v in real.items()}
    orig = bacc_mod.get_activation_tables
    bacc_mod.get_activation_tables = lambda arch: filtered
    try:
        nc.compile()
    finally:
        bacc_mod.get_activation_tables = orig
    return nc


def _get_nc():
    if "nc" not in _CACHE:
        _CACHE["nc"] = _build_nc()
    return _CACHE["nc"]


def make_in_maps(representation, xrecon, xorig):
    rep = np.ascontiguousarray(np.asarray(representation, dtype=np.float32))
    xrec = np.asarray(xrecon, dtype=np.float32).astype(ml_dtypes.bfloat16)
    xorg = np.asarray(xorig, dtype=np.float32).astype(ml_dtypes.bfloat16)
    repb = rep.astype(ml_dtypes.bfloat16)
    RT = np.ascontiguousarray(repb.T)  # (512, 4096) bf16
    in_maps = []
    for c in range(NCORES):
        partner = (c + 4) % 8
        order = [partner, c, (c + 1) % 8, (c + 2) % 8, (c + 3) % 8]
        rt_c = np.concatenate([RT[:, CH * p : CH * (p + 1)] for p in order], axis=1)
        # [d, p, ch, col] -> [ch, p, d, col]  (ch = 512-col chunk index)
        rt_t = np.ascontiguousarray(
            rt_c.reshape(4, 128, 5, 512).transpose(2, 1, 0, 3).reshape(5, 128, 2048)
        )
        in_maps.append(
            {
                "rt": rt_t,
                "xr": np.ascontiguousarray(
                    xrec[CH * c : CH * (c + 1)]
                    .reshape(4, 128, 1024).transpose(1, 0, 2).reshape(128, 4096)
                ),
                "xo": np.ascontiguousarray(
                    xorg[CH * c : CH * (c + 1)]
                    .reshape(4, 128, 1024).transpose(1, 0, 2).reshape(128, 4096)
                ),
                "zi": np.ascontiguousarray(
                    repb[256 * c : 256 * (c + 1)]
                    .reshape(2, 128, D).transpose(1, 0, 2).reshape(128, 1024)
                ),
                "zj": np.ascontiguousarray(
                    repb[2048 + 256 * c : 2048 + 256 * (c + 1)]
                    .reshape(2, 128, D).transpose(1, 0, 2).reshape(128, 1024)
                ),
            }
        )
    return in_maps


def combine_outputs(results):
    """results: list of 8 dicts with out [128,10], orow [128,8], ocol [1,1536]."""
    EXP_DIAG = math.exp(1.0 / TAU)
    denom = np.zeros(TWO_N, dtype=np.float64)
    pos = np.zeros(TWO_N, dtype=np.float64)
    mse = np.zeros(10, dtype=np.float64)
    for c in range(NCORES):
        orow = np.asarray(results[c]["orow"], dtype=np.float64)  # [128, 8]
        # partition p, col rr -> global row 512c + 128rr + p
        rows = orow[:, 0:4].T.reshape(-1)  # [rr*128 + p]
        denom[CH * c : CH * (c + 1)] += rows
        pos[CH * c : CH * (c + 1)] = orow[:, 4:8].T.reshape(-1)
        ocol = np.asarray(results[c]["ocol"], dtype=np.float64).reshape(3, CH)
        for k in range(3):
            m = (c + 1 + k) % NCORES
            denom[CH * m : CH * (m + 1)] += ocol[k]
        bn = np.asarray(results[c]["out"], dtype=np.float64).reshape(128, 5, 2, 6)
        cnt_e, mean_e, m2_e = bn[..., 0], bn[..., 1], bn[..., 2]
        cnt_o, mean_o, m2_o = bn[..., 3], bn[..., 4], bn[..., 5]
        sumsq = (
            m2_e + cnt_e * mean_e**2 + m2_o + cnt_o * mean_o**2
        ).sum(axis=(0, 2))  # [5] per-tile sum of squared diffs
        mse[0:5] += sumsq
    denom -= EXP_DIAG
    closs = (np.log(denom) - pos).sum() / TWO_N
    recon = mse[0:4].sum() / TWO_N
    zrec = mse[4] / N
    loss = recon + closs + zrec
    f = np.float32
    return (f(loss), f(closs), f(recon), f(zrec))


def kernel(representation, xrecon, xorig):
    from concourse.bass_utils import run_bass_kernel_spmd

    nc = _get_nc()
    in_maps = make_in_maps(representation, xrecon, xorig)
    res = run_bass_kernel_spmd(nc, in_maps, core_ids=list(range(NCORES)))
    return combine_outputs(res.results)

